# revision 4
# baseline (speedup 1.0000x reference)
"""Trainium2 Bass kernel v2 for nn_Attention_82403242541756.

Reference semantics (with the dim-0 chunk bug):
  qkv = inputs @ W_qkv + b_qkv                  # [3, 2048, 3072]
  q, k, v = split(qkv, 3, axis=0)               # batch split! q=batch0, k=batch1, v=batch2
  each chunk [1, 2048, 3072] flat-reinterpreted to (3, 16, 2048, 64) = 48 "heads"
  scores softmax (no max trick needed; |scores| < 2.2), ctx, flat-reinterpret, @ W_out + b_out

Sharding (zero communication): core c takes seq rows [256c, 256c+256) of all 3
batch items -> 6 heads/core, each [2048, 64].

v2 design (cost-model-driven; ACT exp is the ~200us/core wall):
  - all-bf16 matmuls (fp8 rejected: random-sign dot products keep the full
    per-element quantization noise, ~3.8%% output error).
  - scores bf16 in [t,s] psum orientation, exp on ACT paced by 2x[128,1024] psums.
  - AV FLIPPED: lhsT = expT s-strided chunk (stationary, ldweights free),
    rhs = vx [t, 64 v cols | 16.0 ones col]: 64+1 streamed cols instead of 2048
    -> halved AV cost, and the softmax denominator accumulates into a psum
    column -> normalize is reciprocal + tensor_scalar (no partition broadcast).
  - ctx [a, (t16,d)] blocks transposed via SBUF->SBUF XBAR DMA (no psum), then
    out-proj FLIPPED too: lhsT = ctxn chunk (stationary), rhs = W_out cols ->
    out [a, j] in natural orientation, full-bandwidth 2KB-row stores.
  - PSUM (8 banks): scores 2x[128,1024] (4) + ctx+den [128,17,64] (3) +
    out-proj quarter ring [128,256] (1).
  - QKV m1-slabs, next-head transposes and prev-head finish pieces are
    interleaved one-per-tt into the head loops so PE never idles >3us
    (cost-model p-state) and ACT never starves.
"""

import sys

sys.path.insert(0, "/opt/trn_rl_repo")

import contextlib

import numpy as np
import ml_dtypes

from concourse import bacc, bass, mybir, tile
from concourse.bass_utils import run_bass_kernel_spmd

BF16 = mybir.dt.bfloat16
F32 = mybir.dt.float32
AF = mybir.ActivationFunctionType
ALU = mybir.AluOpType

P = 128
N_CORES = 8
SEQ = 2048
H = 1024
HEADS = 6
ROWS = 256  # seq rows per core

SC_EFF = float(H) ** -0.5
ONES_VAL = 1.0

_NC_CACHE = {}


def _build():
    nc = bacc.Bacc()

    # xt slab-major: slab s = 2*b + m, [P, slab, k, row]
    xt_e = nc.declare_dram_parameter("xt", [P, 6, 8, 128], BF16, isOutput=False)
    wq_e = nc.declare_dram_parameter("wq", [P, 8, 3072], BF16, isOutput=False)
    bq_e = nc.declare_dram_parameter("bq", [1, 3072], BF16, isOutput=False)
    # out-proj contraction h' = 64*t16 + d; partition p' = (j=t16%2)*64 + d,
    # chunk u = t16//2
    wo_e = nc.declare_dram_parameter("wo", [P, 8, 1024], BF16, isOutput=False)
    bo_e = nc.declare_dram_parameter("bo", [P, 1024], BF16, isOutput=False)
    out_e = nc.declare_dram_parameter("outt", [768, 1024], BF16, isOutput=True)

    with tile.TileContext(nc) as tc:
        with (
            tc.tile_pool(name="dram", bufs=1, space="DRAM") as dp,
            tc.tile_pool(name="w1", bufs=1) as w1p,
            tc.tile_pool(name="qk", bufs=5) as qkp,
            tc.tile_pool(name="expp", bufs=8) as expp,
            tc.tile_pool(name="cn", bufs=10) as cnp,
            tc.tile_pool(name="rrp", bufs=2) as rrp,
            tc.tile_pool(name="ctxn", bufs=2) as ctxnp,
            tc.tile_pool(name="stg", bufs=1) as stgp,
        ):
            yq = dp.tile([12288, 128], BF16)
            yk = dp.tile([12288, 128], BF16)
            yv = dp.tile([12288, 64], BF16)
            yq_v = yq.rearrange("(r j) d -> r j d", j=48)
            yk_v = yk.rearrange("(r j) d -> r j d", j=48)
            yv_v = yv.rearrange("(r j) d -> r (j d)", j=48)

            xt_sb = w1p.tile([P, 6, 8, 128], BF16)
            wq_sb = w1p.tile([P, 8, 3072], BF16)

            def load_xt_slab(s):
                nc.sync.dma_start(xt_sb[:, s], xt_e[:, s])

            # critical ramp order: first slabs + wq stream; bq after nb0
            load_xt_slab(0)
            nc.sync.dma_start(
                wq_sb[:, :, 0:512],
                wq_e[:, :, 0:512],
            )
            bq_small = w1p.tile([1, 3072], BF16)
            nc.sync.dma_start(bq_small[:], bq_e[:])
            bq_sb = w1p.tile([P, 3072], BF16)
            nc.gpsimd.partition_broadcast(bq_sb[:], bq_small[:])
            load_xt_slab(2)
            for nb in range(1, 6):
                nc.sync.dma_start(
                    wq_sb[:, :, 512 * nb : 512 * (nb + 1)],
                    wq_e[:, :, 512 * nb : 512 * (nb + 1)],
                )
            load_xt_slab(4)
            wo_sb = w1p.tile([P, 8, 1024], BF16)
            bo_sb = w1p.tile([P, 1024], BF16)

            def load_wo():
                # artificial dep: keeps the greedy scheduler from hoisting the
                # big wo transfer ahead of the ramp-critical staging writes
                nc.vector.tensor_copy(out=wo_sb[0:1, 0, 0:1], in_=h_qT[0][0][0:1, 0:1])
                nc.sync.dma_start(wo_sb[:], wo_e[:])
                nc.sync.dma_start(bo_sb[:], bo_e[:])

            # vx: [t-part, (head,so) chunk, 64 v cols + ones col]
            vx = w1p.tile([P, 96, 65], BF16)
            nc.vector.memset(vx[:, :, 64:65], ONES_VAL)

            # persistent per-slab staging: pad cols [64:128) zeroed once
            ybqk = []
            for i in range(3):
                t = w1p.tile([P, 48, 128], BF16, name=f"ybqk{i}")
                nc.vector.memset(t[:, :, 64:128], 0.0)
                ybqk.append(t)
            ybv_t = [w1p.tile([P, 3072], BF16, name="ybv0")]
            slab_state = {"qk": 0, "v": 0}

            def emit_qkv_group(ps, b, m, nb, stage):
                """One QKV psum group: slab (b,m) x 512-col block nb -> staging."""
                for k in range(8):
                    nc.tensor.matmul(
                        ps[:, 0:512],
                        lhsT=xt_sb[:, 2 * b + m, k, :],
                        rhs=wq_sb[:, k, 512 * nb : 512 * (nb + 1)],
                        start=(k == 0),
                        stop=(k == 7),
                    )
                if b < 2:
                    nc.vector.tensor_tensor(
                        stage[:, 8 * nb : 8 * (nb + 1), 0:64],
                        ps[:, 0:512].rearrange("p (j d) -> p j d", d=64),
                        bq_sb[:, 512 * nb : 512 * (nb + 1)].rearrange(
                            "p (j d) -> p j d", d=64
                        ),
                        ALU.add,
                    )
                else:
                    nc.vector.tensor_tensor(
                        stage[:, 512 * nb : 512 * (nb + 1)],
                        ps[:, 0:512],
                        bq_sb[:, 512 * nb : 512 * (nb + 1)],
                        ALU.add,
                    )

            def emit_slab_write(b, m, stage, r0, r1):
                if b < 2:
                    dst = (yq_v if b == 0 else yk_v)[
                        128 * m + r0 : 128 * m + r1, :, :
                    ]
                    nc.sync.dma_start(dst, stage[r0:r1])
                else:
                    nc.sync.dma_start(
                        yv_v[128 * m : 128 * (m + 1), :], stage[:]
                    )

            def emit_vx_load(l):
                src = yv.rearrange("(l so p) d -> p (l so) d", p=P, so=16)[
                    :, 16 * l : 16 * (l + 1), :
                ]
                nc.sync.dma_start(vx[:, 16 * l : 16 * (l + 1), 0:64], src)

            def emit_transposes(l):
                qT = qkp.tile([P, SEQ], BF16, tag="qk", name=f"qT{l}")
                nc.sync.dma_start(qT[:], yq[SEQ * l : SEQ * (l + 1), :], transpose=True)
                kT = qkp.tile([P, SEQ], BF16, tag="qk", name=f"kT{l}")
                keng = nc.scalar if l == 0 else nc.sync
                keng.dma_start(kT[:], yk[SEQ * l : SEQ * (l + 1), :], transpose=True)
                return qT, kT

            # ------------- head-finish pieces (spread one per tt) -----------
            # piece 0: reciprocal + normalize (DVE)
            # pieces 1..8: one SBUF->SBUF DMA-transpose each -> ctxn chunk
            # pieces 9..12: one out-proj quarter (8 mms) + bias stage each
            # piece 13: store
            fin_state = {}

            def emit_finish_piece(l, piece, shpool):
                st = fin_state[l]
                ctxps = st["ctxps"]
                if piece == 0:
                    rr = rrp.tile([P, 16, 1], F32, tag="rr", name=f"rr{l}")
                    nc.vector.reciprocal(rr[:, :, 0], st["denps"][:])
                    cpres = []
                    for u in range(8):
                        cpre = cnp.tile([P, P], BF16, tag="cpre", name=f"cp{l}_{u}")
                        nc.vector.tensor_tensor(
                            cpre.rearrange("p (j d) -> p j d", d=64),
                            ctxps[:, 2 * u : 2 * u + 2, :],
                            rr[:, 2 * u : 2 * u + 2, :].to_broadcast([P, 2, 64]),
                            ALU.mult,
                        )
                        cpres.append(cpre)
                    st["cpres"] = cpres
                    st["ctxn"] = ctxnp.tile(
                        [P, 8, P], BF16, tag="ctxn", name=f"ctxn{l}"
                    )
                elif piece <= 8:
                    u = piece - 1
                    eng = nc.scalar if (l == 5 and u % 2 == 1) else nc.sync
                    eng.dma_start(
                        st["ctxn"][:, u, :], st["cpres"][u][:], transpose=True
                    )
                elif piece <= 12:
                    q = piece - 9
                    if q == 0:
                        st["stg"] = stgp.tile([P, 1024], BF16, tag="stg", name=f"st{l}")
                    if l == 5:
                        # scores are done: use a freed scps tile per 2 quarters
                        if q % 2 == 0:
                            st["opt"] = scps.tile(
                                [P, 1024], F32, tag="sc", name=f"opt{l}_{q}"
                            )
                        ops = st["opt"][:, 256 * (q % 2) : 256 * (q % 2 + 1)]
                    else:
                        opst = shpool.tile([P, 512], F32, tag="misc", name=f"op{l}_{q}")
                        ops = opst[:, 0:256]
                    for u in range(8):
                        nc.tensor.matmul(
                            ops[:],
                            lhsT=st["ctxn"][:, u, :],
                            rhs=wo_sb[:, u, 256 * q : 256 * (q + 1)],
                            start=(u == 0),
                            stop=(u == 7),
                        )
                    nc.vector.tensor_tensor(
                        st["stg"][:, 256 * q : 256 * (q + 1)],
                        ops[:],
                        bo_sb[:, 256 * q : 256 * (q + 1)],
                        ALU.add,
                    )
                else:  # store
                    nc.gpsimd.dma_start(
                        out_e[128 * l : 128 * (l + 1), :], st["stg"][:]
                    )

            # ------------------- head tt-loop -------------------------------
            def emit_head_loop(l, qT, kT, scps, ctxpsp, shpool, interleave):
                pend = []
                st = {}
                fin_state[l] = st

                def emit_av():
                    tt, expT = pend.pop(0)
                    # For head 0, push AV matmuls later in scheduler priority:
                    # they wait on vx0 (late v-path) and must not be ordered
                    # ahead of score matmuls, which would stall the exp wall.
                    prio_orig = None
                    if l == 0:
                        prio_orig = tc.cur_priority
                        tc.cur_priority = prio_orig + 200
                    if "ctxps" not in st:
                        st["ctxps"] = ctxpsp.tile(
                            [P, 16, 64], F32, tag="ctx", name=f"ctx{l}"
                        )
                        st["denps"] = denpool.tile(
                            [P, 16], F32, tag="den", name=f"den{l}"
                        )
                    ctxps = st["ctxps"]
                    denps = st["denps"]
                    expT_r = expT.rearrange("p (a s) -> p s a", s=16)
                    # one psum accumulation group per 2KB bank: bank0 = t16
                    # 0..7, bank1 = t16 8..15, bank2 = denom column
                    for t16 in range(16):
                        lhsT = expT_r[:, t16, :]
                        nc.tensor.matmul(
                            ctxps[:, t16, :],
                            lhsT=lhsT,
                            rhs=vx[:, 16 * l + tt, 0:64],
                            start=(tt == 0 and t16 % 8 == 0),
                            stop=(tt == 15 and t16 % 8 == 7),
                            skip_group_check=True,
                        )
                        nc.tensor.matmul(
                            denps[:, t16 : t16 + 1],
                            lhsT=lhsT,
                            rhs=vx[:, 16 * l + tt, 64:65],
                            start=(tt == 0 and t16 == 0),
                            stop=(tt == 15 and t16 == 15),
                            skip_group_check=True,
                        )
                    if prio_orig is not None:
                        tc.cur_priority = prio_orig

                for tt in range(16):
                    expT = expp.tile([P, SEQ], BF16, tag="expT", name=f"ex{l}_{tt}")
                    for hh in range(2):
                        sc = scps.tile(
                            [P, 1024], F32, name=f"sc{l}_{tt}_{hh}", tag="sc"
                        )
                        for s2 in range(2):
                            s0 = 1024 * hh + 512 * s2
                            nc.tensor.matmul(
                                sc[:, 512 * s2 : 512 * (s2 + 1)],
                                lhsT=kT[0:64, 128 * tt : 128 * (tt + 1)],
                                rhs=qT[0:64, s0 : s0 + 512],
                                start=True,
                                stop=True,
                            )
                        nc.scalar.activation(
                            expT[:, 1024 * hh : 1024 * (hh + 1)],
                            sc[:],
                            AF.Exp,
                            scale=SC_EFF,
                        )
                    pend.append((tt, expT))
                    look = 6 if l == 0 else (4 if l == 3 else (2 if l < 5 else 0))
                    if len(pend) > look:
                        emit_av()
                    if tt in interleave:
                        for fn in interleave[tt]:
                            fn()
                while pend:
                    emit_av()

            # ---------------- phase 1: m0 slabs (heads 0-2 data) ------------
            es1 = contextlib.ExitStack()
            ps_init = es1.enter_context(
                tc.tile_pool(name="psi", bufs=4, space="PSUM", side="right")
            )

            def psi_group(b, m, nb, stage):
                ps = ps_init.tile([P, 512], F32, name=f"yps{b}{m}{nb}", tag="yps")
                emit_qkv_group(ps, b, m, nb, stage)

            # PE pre-warm: ~3.5us of junk matmuls so phase-1 GEMMs run at the
            # warm p-state (cost model halves matmul speed after idle)
            warm = ps_init.tile([P, 512], F32, name="warm", tag="yps")
            for i in range(8):
                nc.tensor.matmul(
                    warm[:, 0:256],
                    lhsT=xt_sb[:, 0, 0, :],
                    rhs=xt_sb[:, 0, 0:2, :],
                    start=(i == 0),
                    stop=(i == 7),
                )
            # interleave b0/b1 groups per nb so both GEMMs hide under the wq
            # transfer stream; write rows 0:43 first so head-0 transposes fire
            # right after the last add
            for nb in range(6):
                psi_group(0, 0, nb, ybqk[0])
                psi_group(1, 0, nb, ybqk[1])
            emit_slab_write(0, 0, ybqk[0], 0, 43)
            emit_slab_write(1, 0, ybqk[1], 0, 43)
            h_qT = {0: emit_transposes(0)}
            for nb in range(6):
                psi_group(2, 0, nb, ybv_t[0])
            emit_slab_write(2, 0, ybv_t[0], 0, 128)
            emit_vx_load(0)
            emit_slab_write(0, 0, ybqk[0], 43, 128)
            emit_slab_write(1, 0, ybqk[1], 43, 128)
            h_qT[1] = emit_transposes(1)
            load_xt_slab(1)
            load_xt_slab(3)
            load_xt_slab(5)
            es1.close()

            # ---------------- phase 2: head loops ---------------------------
            with (
                tc.tile_pool(name="scps", bufs=2, space="PSUM", side="left") as scps,
                tc.tile_pool(name="ctxps", bufs=1, space="PSUM", side="right") as ctxpsp,
                tc.tile_pool(name="den", bufs=1, space="PSUM", side="right") as denpool,
                tc.tile_pool(name="misc", bufs=1, space="PSUM", side="right") as shpool,
            ):

                def ilv_qkv(b, m, nb):
                    def fn():
                        stage = ybqk[nb_stage[(b, m)]] if b < 2 else ybv_t[0]
                        ps = shpool.tile(
                            [P, 512], F32, name=f"yq{b}{m}{nb}", tag="misc"
                        )
                        emit_qkv_group(ps, b, m, nb, stage)

                    return fn

                def ilv_write(b, m):
                    def fn():
                        if b < 2:
                            emit_slab_write(b, m, ybqk[nb_stage[(b, m)]], 0, 128)
                        else:
                            emit_slab_write(b, m, ybv_t[0], 0, 128)

                    return fn

                # staging assignment for m1 slabs: reuse ring slots
                nb_stage = {(0, 1): 2, (1, 1): 0, (2, 1): None}

                def ilv_transp(l):
                    def fn():
                        h_qT[l] = emit_transposes(l)

                    return fn

                def ilv_finish(l, piece):
                    def fn():
                        emit_finish_piece(l, piece, shpool)

                    return fn

                def mk_interleave(l):
                    iv = {}
                    if l > 0:
                        # finish pieces of head l-1: norm at tt1, transposes
                        # tt2-9, out-proj quarters tt11-14, store tt15
                        iv.setdefault(1, []).append(ilv_finish(l - 1, 0))
                        for u in range(8):
                            iv.setdefault(2 + u, []).append(ilv_finish(l - 1, 1 + u))
                        for q in range(4):
                            iv.setdefault(11 + q, []).append(ilv_finish(l - 1, 9 + q))
                        iv.setdefault(15, []).append(ilv_finish(l - 1, 13))
                    # m1 QKV slabs: one group per 2 tts, spread over heads 0-2
                    if l == 0:
                        for g, tt in enumerate(range(1, 13, 2)):
                            iv.setdefault(tt, []).append(ilv_qkv(0, 1, g))
                        iv.setdefault(13, []).append(ilv_write(0, 1))
                        iv.setdefault(2, []).append(lambda: emit_vx_load(1))
                        iv.setdefault(4, []).append(lambda: emit_vx_load(2))
                        iv.setdefault(6, []).append(load_wo)
                        iv.setdefault(14, []).append(ilv_transp(2))
                    elif l == 1:
                        for g, tt in enumerate(range(1, 12, 2)):
                            iv.setdefault(tt, []).append(ilv_qkv(1, 1, g))
                        iv.setdefault(13, []).append(ilv_write(1, 1))
                    elif l == 2:
                        iv.setdefault(2, []).append(ilv_transp(3))
                        for g, tt in enumerate(range(5, 16, 2)):
                            iv.setdefault(tt, []).append(ilv_qkv(2, 1, g))
                        iv.setdefault(14, []).append(ilv_transp(4))
                    elif l == 3:
                        iv.setdefault(1, []).append(ilv_write(2, 1))
                        iv.setdefault(2, []).append(lambda: emit_vx_load(3))
                        iv.setdefault(3, []).append(lambda: emit_vx_load(4))
                        iv.setdefault(5, []).append(lambda: emit_vx_load(5))
                        iv.setdefault(14, []).append(ilv_transp(5))
                    return iv

                for l in range(HEADS):
                    qT, kT = h_qT[l]
                    emit_head_loop(l, qT, kT, scps, ctxpsp, shpool, mk_interleave(l))
                # tail: tightly pipelined finish for the last head
                st = fin_state[5]
                ctxps = st["ctxps"]
                rr = rrp.tile([P, 16, 1], F32, tag="rr", name="rr5")
                nc.vector.reciprocal(rr[:, :, 0], st["denps"][:])
                st["ctxn"] = ctxnp.tile([P, 8, P], BF16, tag="ctxn", name="ctxn5")
                for u in range(8):
                    cpre = cnp.tile([P, P], BF16, tag="cpre", name=f"cp5_{u}")
                    nc.vector.tensor_tensor(
                        cpre.rearrange("p (j d) -> p j d", d=64),
                        ctxps[:, 2 * u : 2 * u + 2, :],
                        rr[:, 2 * u : 2 * u + 2, :].to_broadcast([P, 2, 64]),
                        ALU.mult,
                    )
                    eng = nc.scalar if u % 2 == 1 else nc.sync
                    eng.dma_start(st["ctxn"][:, u, :], cpre[:], transpose=True)
                stg5 = stgp.tile([P, 1024], BF16, tag="stg", name="st5")
                opts = [
                    scps.tile([P, 1024], F32, tag="sc", name=f"opt5_{i}")
                    for i in range(2)
                ]
                for u in range(8):
                    for q in range(4):
                        nc.tensor.matmul(
                            opts[q // 2][:, 256 * (q % 2) : 256 * (q % 2 + 1)],
                            lhsT=st["ctxn"][:, u, :],
                            rhs=wo_sb[:, u, 256 * q : 256 * (q + 1)],
                            start=(u == 0 and q % 2 == 0),
                            stop=(u == 7 and q % 2 == 1),
                            skip_group_check=True,
                        )
                for q in range(4):
                    nc.vector.tensor_tensor(
                        stg5[:, 256 * q : 256 * (q + 1)],
                        opts[q // 2][:, 256 * (q % 2) : 256 * (q % 2 + 1)],
                        bo_sb[:, 256 * q : 256 * (q + 1)],
                        ALU.add,
                    )
                    nc.sync.dma_start(
                        out_e[128 * 5 : 128 * 6, 256 * q : 256 * (q + 1)],
                        stg5[:, 256 * q : 256 * (q + 1)],
                    )

    nc.finalize()
    return nc


def _get_nc():
    if "nc" not in _NC_CACHE:
        _NC_CACHE["nc"] = _build()
    return _NC_CACHE["nc"]


def make_in_maps(inputs, W_qkv, b_qkv, W_out, b_out):
    bf = ml_dtypes.bfloat16
    x = np.asarray(inputs, dtype=np.float32)
    Wq = np.asarray(W_qkv, dtype=np.float32)
    bq = np.asarray(b_qkv, dtype=np.float32)
    Wo = np.asarray(W_out, dtype=np.float32)
    bo = np.asarray(b_out, dtype=np.float32)

    wq_s = np.ascontiguousarray(Wq.reshape(8, P, 3072).transpose(1, 0, 2)).astype(bf)
    bq_s = np.ascontiguousarray(bq[None, :]).astype(bf)
    # wo: row h' = 64*t16 + d -> [p'=(64j+d), u, jcol] with t16 = 2u+j
    wo_r = Wo.reshape(16, 64, 1024)  # [t16, d, j]
    wo_s = np.empty((P, 8, 1024), dtype=np.float32)
    for u in range(8):
        for j in range(2):
            wo_s[64 * j : 64 * (j + 1), u, :] = wo_r[2 * u + j]
    wo_s = np.ascontiguousarray(wo_s).astype(bf)
    bo_s = np.ascontiguousarray(np.broadcast_to(bo[None, :], (P, 1024))).astype(bf)

    in_maps = []
    for c in range(N_CORES):
        xc = x[:, ROWS * c : ROWS * (c + 1), :]  # [3, 256, 1024]
        # [1024, 768] -> slabs s=2b+m of 128 rows -> [P, 6, 8, 128]
        xt = (
            xc.transpose(2, 0, 1)
            .reshape(8, P, 6, 128)
            .transpose(1, 2, 0, 3)
        )
        in_maps.append(
            {
                "xt": np.ascontiguousarray(xt).astype(bf),
                "wq": wq_s,
                "bq": bq_s,
                "wo": wo_s,
                "bo": bo_s,
            }
        )
    return in_maps


def kernel(inputs, W_qkv, b_qkv, W_out, b_out, _trace=False, _trace_kwargs=None):
    in_maps = make_in_maps(inputs, W_qkv, b_qkv, W_out, b_out)
    nc = _get_nc()
    kw = {}
    if _trace:
        kw["trace"] = True
        if _trace_kwargs:
            kw.update(_trace_kwargs)
    res = run_bass_kernel_spmd(nc, in_maps, core_ids=list(range(N_CORES)), **kw)
    outs = res.results

    out = np.empty((6144, 1024), dtype=np.float32)
    for c in range(N_CORES):
        out[768 * c : 768 * (c + 1), :] = np.asarray(outs[c]["outt"], dtype=np.float32)
    if _trace:
        kernel.last_result = res
    return out.reshape(3, SEQ, H)
